# revision 1
# baseline (speedup 1.0000x reference)
"""Trainium2 Bass kernel v2 for nn_Block_24532853194876.

Feature-major design:
  - x uploaded pre-transposed [D, TP]; LN stats via ACT-square + PE
    ones-matmul reductions; mean removal folded into matmuls as rank-1
    accumulation steps; rstd folded into rope fields (q/k), per-token
    STT scale (v), and broadcast fields (proj/w3 outputs).
  - q/k/od kept SBUF-resident in bf16; v via DRAM in a per-head
    65-col-stride layout with a built-in ones column for softmax sums.
  - softmax: exp on ACT (PSUM->bf16), exp(bias) multiplied in bf16,
    normalization deferred to phase D via a reciprocal field built with
    selector matmuls.
  - MLP in bf16; ffn-LN stats computed in G per token-slice just in time.
Output is feature-major outT [D, TP]; host transposes.
"""
import sys
sys.path.insert(0, '/opt/trn_rl_repo')
import numpy as np

B, NT, D, H, HD, HID = 64, 257, 1024, 16, 64, 4096
NCORES = 8
BL = B // NCORES
T = BL * NT                 # 2056
TP = 2176                   # 17*128
NCH = TP // 128             # 17
KD = D // 128               # 8
MH = HID // 128             # 32
QP = 260
LN_EPS = 1e-5
WS = 16
SCALE = HD ** -0.5
TOK_SLICES = [(0, 512), (512, 512), (1024, 512), (1536, 384), (1920, 256)]
KCH = [(0, 128), (128, 128), (256, 1)]
VROW = H * 65               # 1040

_CACHE = {}


def _rel_pos_index():
    ch, cw = np.meshgrid(np.arange(WS), np.arange(WS), indexing='ij')
    flat = np.stack([ch.reshape(-1), cw.reshape(-1)])
    rel = flat[:, :, None] - flat[:, None, :]
    rel = rel.transpose(1, 2, 0).astype(np.int64)
    rel[:, :, 0] += WS - 1
    rel[:, :, 1] += WS - 1
    rel[:, :, 0] *= 2 * WS - 1
    nrd = (2 * WS - 1) * (2 * WS - 1) + 3
    idx = np.zeros((NT, NT), dtype=np.int64)
    idx[1:, 1:] = rel.sum(-1)
    idx[0, 0:] = nrd - 3
    idx[0:, 0] = nrd - 2
    idx[0, 0] = nrd - 1
    return idx


def _fix_wait_limits(nc, mybir, max_waits=1):
    for f in nc.m.functions:
        for bb in f.blocks:
            insts = bb.instructions
            i = 0
            while i < len(insts):
                inst = insts[i]
                si = inst.sync_info
                if si and si.on_wait and len(si.on_wait) > max_waits:
                    extra = si.on_wait[:-max_waits]
                    inst.sync_info.on_wait = si.on_wait[-max_waits:]
                    for j, w in enumerate(extra):
                        nop = mybir.InstNoOp(
                            name=f"{inst.name}-ws{j}", engine=inst.engine,
                            ins=[], outs=[],
                            sync_info=mybir.SyncInfo(on_wait=[w], on_update=[]),
                        )
                        insts.insert(i, nop)
                        i += 1
                i += 1


def build_module():
    import os
    DBG = os.environ.get('KDBG') == '1'
    REP = int(os.environ.get('KREPEAT') or '1')
    key = ('v2', DBG, REP)
    if key in _CACHE:
        return _CACHE[key]
    import concourse.bass as bass
    import concourse.mybir as mybir
    import concourse.tile as tile

    F32 = mybir.dt.float32
    F32R = mybir.dt.float32r
    BF16 = mybir.dt.bfloat16
    AF = mybir.ActivationFunctionType
    ALU = mybir.AluOpType

    nc = bass.Bass()
    P = lambda name, shape, dt=F32: nc.declare_dram_parameter(
        name, shape, dt, isOutput=False)

    xT_d = P("xT", [D, TP])                 # raw x transposed, pad zeros
    wqT = P("wqT", [D, D])                  # [in, out] effective (g1, scale)
    wkT = P("wkT", [D, D])
    wvT = P("wvT", [D, D])
    wpT_d = P("wpT", [D, D], BF16)
    w1T_d = P("w1T", [D, HID], BF16)
    w2T_d = P("w2T", [D, HID], BF16)
    w3T_d = P("w3T", [HID, D], BF16)
    cqn = P("cqn", [1, D])                  # -colsum(wqT)
    ckn = P("ckn", [1, D])
    cvn = P("cvn", [1, D])
    cpn = P("cpn", [1, D], BF16)
    c3n = P("c3n", [1, D], BF16)
    qb_c = P("qb_c", [128, KD])
    kb_c = P("kb_c", [128, KD])
    pb_c = P("pb_c", [128, KD])
    w1b_c = P("w1b_c", [128, MH])
    w2b_c = P("w2b_c", [128, MH])
    w3b_c = P("w3b_c", [128, KD])
    vbr = P("vbr", [1, D], BF16)
    cosr = P("cosr", [128, TP], BF16)
    sinr = P("sinr", [128, TP], BF16)
    r2T_d = P("r2T", [128, 128], BF16)
    expb_d = P("expb", [H, 3, 128, QP], BF16)
    sel_d = P("sel", [KD, 16, 128])
    onesc_d = P("onesc", [128, 1])
    onesr_d = P("onesr", [1, 128])
    onespat_d = P("onespat", [128, 16], BF16)
    mask_d = P("maskr", [1, TP])
    out = nc.declare_dram_parameter("out", [D, TP], F32, isOutput=True)

    if DBG:
        S = lambda name, shape, dt=F32: nc.declare_dram_parameter(
            name, shape, dt, isOutput=True)
        dbg_q = S("dbg_q", [D, TP], BF16)
        dbg_od = S("dbg_od", [D, TP], BF16)
        dbg_sums = S("dbg_sums", [16, TP])
        dbg_x2 = S("dbg_x2", [D, TP])
        dbg_mu = S("dbg_mu", [1, TP])
        dbg_rs = S("dbg_rs", [1, TP])
        dbg_xh2 = S("dbg_xh2", [D, TP], BF16)
    vd = nc.dram_tensor("vd", [TP, VROW], BF16)
    rs_stage = nc.dram_tensor("rs_stage", [1, TP], F32)
    hd = nc.dram_tensor("hd", [HID, TP], BF16)
    x2d = nc.dram_tensor("x2d", [D, TP], F32)

    with tile.TileContext(nc) as tc, \
         nc.allow_low_precision(reason="intentional f32r/bf16 pipeline"):
      for _rep in range(REP):
        consts_cm = tc.tile_pool(name="consts", bufs=1)
        consts = consts_cm.__enter__()
        ones_c = consts.tile([128, 1], F32R)
        nc.sync.dma_start(out=ones_c, in_=onesc_d[:, :].bitcast(F32R))
        ones_cb = consts.tile([128, 1], BF16)
        onesf_t = consts.tile([128, 1], F32)
        nc.vector.memset(onesf_t, 1.0)
        nc.vector.tensor_copy(ones_cb, onesf_t)
        ones_r = consts.tile([1, 128], F32R)
        nc.sync.dma_start(out=ones_r, in_=onesr_d[:, :].bitcast(F32R))
        eps1_t = consts.tile([1, 1], F32)
        nc.vector.memset(eps1_t, LN_EPS)

        rowsA_cm = tc.tile_pool(name="rowsA", bufs=1)
        rowsA = rowsA_cm.__enter__()
        murow = rowsA.tile([1, TP], F32R, name="murow")
        rstok = rowsA.tile([128, NCH], F32, name="rstok")
        sums_t = rowsA.tile([16, TP], F32, name="sums_t")
        rqk_cm = tc.tile_pool(name="rqk", bufs=1, side="right")
        rqk = rqk_cm.__enter__()
        qT = [rqk.tile([128, TP], BF16, name=f"qT{k}", tag=f"qT{k}")
              for k in range(KD)]
        kT = [rqk.tile([128, TP], BF16, name=f"kT{k}", tag=f"kT{k}")
              for k in range(KD)]
        rfld_cm = tc.tile_pool(name="rfld", bufs=1, side="right")
        rfld = rfld_cm.__enter__()
        rsrow = rfld.tile([1, TP], F32R, name="rsrow")
        mask_t = rfld.tile([1, TP], F32, name="mask_t")
        nc.sync.dma_start(out=mask_t, in_=mask_d[:, :])
        cosf = rfld.tile([128, TP], BF16, name="cosf")
        sinf = rfld.tile([128, TP], BF16, name="sinf")

        def ln_rows(pool, ps_sx, ps_sxx, mu_dst, rs_dst, toff, tlen, ndim,
                    mask=None, tagp="", bufs_note=None):
            """mu/rstd rows for one token slice from sum/sumsq psum tiles."""
            nc.scalar.activation(out=mu_dst[:, toff:toff+tlen],
                                 in_=ps_sx[:, :tlen], func=AF.Copy,
                                 scale=1.0/ndim)
            ex2 = pool.tile([1, 512], F32, tag=tagp+"ex2")
            nc.scalar.activation(out=ex2[:, :tlen], in_=ps_sxx[:, :tlen],
                                 func=AF.Copy, scale=1.0/ndim)
            mu2 = pool.tile([1, 512], F32, tag=tagp+"mu2")
            nc.vector.tensor_mul(mu2[:, :tlen], mu_dst[:, toff:toff+tlen],
                                 mu_dst[:, toff:toff+tlen])
            var = pool.tile([1, 512], F32, tag=tagp+"var")
            nc.vector.tensor_sub(var[:, :tlen], ex2[:, :tlen], mu2[:, :tlen])
            sd = pool.tile([1, 512], F32, tag=tagp+"sd")
            nc.scalar.activation(out=sd[:, :tlen], in_=var[:, :tlen],
                                 func=AF.Sqrt, bias=eps1_t[:, 0:1], scale=1.0)
            if mask is None:
                nc.vector.reciprocal(rs_dst[:, toff:toff+tlen], sd[:, :tlen])
            else:
                rcp = pool.tile([1, 512], F32, tag=tagp+"rcp")
                nc.vector.reciprocal(rcp[:, :tlen], sd[:, :tlen])
                nc.vector.tensor_mul(rs_dst[:, toff:toff+tlen], rcp[:, :tlen],
                                     mask[:, toff:toff+tlen])

        # ================= A: xT in + norm1 stats =================
        resA_cm = tc.tile_pool(name="resA", bufs=1)
        resA = resA_cm.__enter__()
        xT = [resA.tile([128, TP], F32R, name=f"xT{k}", tag=f"xT{k}")
              for k in range(KD)]
        _qeng = [nc.sync, nc.gpsimd, nc.scalar]
        for k in range(KD):
            _qeng[k % 3].dma_start(out=xT[k],
                                   in_=xT_d[k*128:(k+1)*128, :].bitcast(F32R))
        with tc.tile_pool(name="pa", bufs=3) as pa, \
             tc.tile_pool(name="pra", bufs=1) as pra, \
             tc.tile_pool(name="psa", bufs=2, space="PSUM") as psa:
            for (toff, tlen) in TOK_SLICES:
                ps_sx = psa.tile([1, 512], F32, tag="a_sx")
                ps_sxx = psa.tile([1, 512], F32, tag="a_sxx")
                for k in range(KD):
                    nc.tensor.matmul(ps_sx[:, :tlen], ones_c,
                                     xT[k][:, toff:toff+tlen],
                                     start=(k == 0), stop=(k == KD-1))
                    sq = pa.tile([128, 512], F32R, tag="a_sq")
                    nc.scalar.activation(out=sq[:, :tlen],
                                         in_=xT[k][:, toff:toff+tlen],
                                         func=AF.Square)
                    nc.tensor.matmul(ps_sxx[:, :tlen], ones_c, sq[:, :tlen],
                                     start=(k == 0), stop=(k == KD-1))
                ln_rows(pra, ps_sx, ps_sxx, murow, rsrow, toff, tlen, D,
                        mask=mask_t, tagp="a_")
            if DBG:
                nc.sync.dma_start(out=dbg_mu[:, :], in_=murow[:, :].bitcast(F32))
                nc.sync.dma_start(out=dbg_rs[:, :], in_=rsrow[:, :].bitcast(F32))
            # per-token rstd columns (for v): bounce via DRAM, transposed read
            nc.sync.dma_start(out=rs_stage[0:1, :],
                              in_=rsrow[0:1, :].bitcast(F32))
            nc.sync.dma_start(
                out=rstok,
                in_=rs_stage[0:1, :].rearrange("o (c p) -> (o p) c", p=128))

        # ============ rope fields cos'=cos*rstd, sin'=sin*rstd ============
        with tc.tile_pool(name="pb0", bufs=2) as pb0, \
             tc.tile_pool(name="psb0", bufs=2, space="PSUM") as psb0:
            cos_raw = pb0.tile([128, TP], BF16, tag="cos_raw")
            sin_raw = pb0.tile([128, TP], BF16, tag="sin_raw")
            nc.sync.dma_start(out=cos_raw, in_=cosr[:, :])
            nc.sync.dma_start(out=sin_raw, in_=sinr[:, :])
            for (toff, tlen) in TOK_SLICES:
                ps_b = psb0.tile([128, 512], F32, tag="rs_bc")
                nc.tensor.matmul(ps_b[:, :tlen], ones_r,
                                 rsrow[:, toff:toff+tlen], start=True, stop=True)
                nc.vector.tensor_mul(cosf[:, toff:toff+tlen],
                                     cos_raw[:, toff:toff+tlen], ps_b[:, :tlen])
                nc.vector.tensor_mul(sinf[:, toff:toff+tlen],
                                     sin_raw[:, toff:toff+tlen], ps_b[:, :tlen])

        # ================= B1: q, k (feature-major, rope) =================
        with tc.tile_pool(name="pb1", bufs=3) as pb1, \
             tc.tile_pool(name="pw1", bufs=2) as pw1, \
             tc.tile_pool(name="pwc", bufs=1) as pwc, \
             tc.tile_pool(name="psb1", bufs=4, space="PSUM") as psb1, \
             tc.tile_pool(name="psb2", bufs=3, space="PSUM") as psb2:
            r2t = pwc.tile([128, 128], BF16)
            nc.sync.dma_start(out=r2t, in_=r2T_d[:, :])
            qb_t = pwc.tile([128, KD], F32)
            kb_t = pwc.tile([128, KD], F32)
            nc.sync.dma_start(out=qb_t, in_=qb_c[:, :])
            nc.sync.dma_start(out=kb_t, in_=kb_c[:, :])
            cq_t = pwc.tile([1, D], F32R, tag="cq_t")
            ck_t = pwc.tile([1, D], F32R, tag="ck_t")
            nc.sync.dma_start(out=cq_t, in_=cqn[:, :].bitcast(F32R))
            nc.sync.dma_start(out=ck_t, in_=ckn[:, :].bitcast(F32R))
            for (wsrc, bcol, ccol, dst) in (
                    (wqT, qb_t, cq_t, qT), (wkT, kb_t, ck_t, kT)):
                for m in range(KD):
                    wma = pw1.tile([128, D], F32R, tag="wma")
                    _we = (nc.sync, nc.gpsimd, nc.scalar)[m % 3]
                    _we.dma_start(
                        out=wma.rearrange("p (g c) -> p g c", g=KD),
                        in_=wsrc[:, m*128:(m+1)*128].bitcast(F32R).rearrange(
                            "(g p) c -> p g c", p=128))
                    wm = [wma[:, k*128:(k+1)*128] for k in range(KD)]
                    for (toff, tlen) in TOK_SLICES:
                        ps = psb1.tile([128, 512], F32, tag="qk_ps")
                        for k in range(KD):
                            nc.tensor.matmul(ps[:, :tlen], wm[k],
                                             xT[k][:, toff:toff+tlen],
                                             start=(k == 0), stop=False)
                        nc.tensor.matmul(ps[:, :tlen],
                                         ccol[:, m*128:(m+1)*128],
                                         murow[:, toff:toff+tlen],
                                         start=False, stop=True)
                        qtmp = pb1.tile([128, 512], BF16, tag="qk_qtmp")
                        nc.scalar.activation(out=qtmp[:, :tlen],
                                             in_=ps[:, :tlen], func=AF.Identity,
                                             bias=bcol[:, m:m+1], scale=1.0)
                        ps2 = psb2.tile([128, 512], F32, tag="qk_ps2")
                        nc.tensor.matmul(ps2[:, :tlen], r2t, qtmp[:, :tlen],
                                         start=True, stop=True)
                        t1 = pb1.tile([128, 512], BF16, tag="qk_t1")
                        nc.vector.tensor_mul(t1[:, :tlen], ps2[:, :tlen],
                                             sinf[:, toff:toff+tlen])
                        t2 = pb1.tile([128, 512], BF16, tag="qk_t2")
                        nc.vector.tensor_mul(t2[:, :tlen], qtmp[:, :tlen],
                                             cosf[:, toff:toff+tlen])
                        nc.vector.tensor_add(dst[m][:, toff:toff+tlen],
                                             t1[:, :tlen], t2[:, :tlen])
            if DBG:
                for k in range(KD):
                    nc.sync.dma_start(out=dbg_q[k*128:(k+1)*128, :], in_=qT[k])
        rfld_cm.__exit__(None, None, None)

        # ====== C sbuf pools open early (right side; overlap with B2) ======
        pc_cm = tc.tile_pool(name="pc", bufs=6, side="right")
        pc = pc_cm.__enter__()
        pcb_cm = tc.tile_pool(name="pcb", bufs=2, side="right")
        pcb = pcb_cm.__enter__()

        # ================= B2: v -> vd DRAM =================
        with tc.tile_pool(name="pb2", bufs=3) as pb2, \
             tc.tile_pool(name="pw2", bufs=1) as pw2, \
             tc.tile_pool(name="pw2h", bufs=1) as pw2h, \
             tc.tile_pool(name="psv", bufs=3, space="PSUM") as psv:

            vb_b = pw2.tile([128, D], BF16)
            nc.sync.dma_start(out=vb_b, in_=vbr[0:1, :].partition_broadcast(128))
            onespat_t = pw2.tile([128, 16], BF16)
            nc.sync.dma_start(out=onespat_t, in_=onespat_d[:, :])
            for ns in range(2):
                cv_t = pw2h.tile([1, 512], F32R, tag="cv_t")
                nc.sync.dma_start(
                    out=cv_t, in_=cvn[0:1, ns*512:(ns+1)*512].bitcast(F32R))
                wva = pw2h.tile([128, KD * 512], F32R, tag="wva")
                nc.sync.dma_start(
                    out=wva.rearrange("p (g c) -> p g c", g=KD),
                    in_=wvT[:, ns*512:(ns+1)*512].bitcast(F32R).rearrange(
                        "(g p) c -> p g c", p=128))
                wv = [wva[:, k*512:(k+1)*512] for k in range(KD)]
                for c in range(NCH):
                    ps = psv.tile([128, 512], F32, tag="v_ps")
                    for k in range(KD):
                        nc.tensor.matmul(ps, xT[k][:, c*128:(c+1)*128],
                                         wv[k],
                                         start=(k == 0), stop=False)
                    nc.tensor.matmul(ps, murow[:, c*128:(c+1)*128],
                                     cv_t, start=False, stop=True)
                    vs = pb2.tile([128, 520], BF16, tag="v_t")
                    nc.vector.tensor_copy(
                        vs.rearrange("p (g c) -> p g c", g=8)[:, :, 64:65],
                        onespat_t[:, ns*8:(ns+1)*8].unsqueeze(-1))
                    nc.vector.scalar_tensor_tensor(
                        out=vs.rearrange("p (g c) -> p g c", g=8)[:, :, 0:64],
                        in0=ps.rearrange("p (g c) -> p g c", g=8),
                        scalar=rstok[:, c:c+1],
                        in1=vb_b[:, ns*512:(ns+1)*512].rearrange(
                            "p (g c) -> p g c", g=8),
                        op0=ALU.mult, op1=ALU.add)
                    nc.gpsimd.dma_start(
                        out=vd[c*128:(c+1)*128, ns*520:(ns+1)*520], in_=vs)
        resA_cm.__exit__(None, None, None)

        # ================= C: attention =================
        psc_cm = tc.tile_pool(name="psc", bufs=5, space="PSUM", side="right")
        psc = psc_cm.__enter__()
        psd_cm = tc.tile_pool(name="psd", bufs=3, space="PSUM", side="right")
        psd = psd_cm.__enter__()
        rod_cm = tc.tile_pool(name="rod", bufs=1)
        rod = rod_cm.__enter__()
        odT = [rod.tile([128, TP], BF16, name=f"odT{k}", tag=f"odT{k}")
               for k in range(KD)]
        peb_cm = tc.tile_pool(name="peb", bufs=1)
        peb = peb_cm.__enter__()
        if True:
            nc.vector.memset(sums_t, 1.0)
            for k in range(KD):
                nc.vector.memset(odT[k][:, T:TP], 0.0)
            ebts = {}
            for h in range(H):
                for kc, (koff, klen) in enumerate(KCH):
                    e = peb.tile([128, QP], BF16, name=f"eb{h}_{kc}",
                                 tag=f"eb{h}_{kc}")
                    nc.gpsimd.dma_start(
                        out=e[:klen, :], in_=expb_d[h, kc, :klen, :])
                    ebts[(h, kc)] = e
            for b in range(BL):
                t0 = b * NT
                vts = []
                for kc, (koff, klen) in enumerate(KCH):
                    vt = pcb.tile([128, VROW], BF16, tag=f"at_v{kc}")
                    nc.sync.dma_start(out=vt[:klen, :],
                                      in_=vd[t0+koff:t0+koff+klen, :])
                    vts.append(vt)
                for h in range(H):
                    kch, row0 = h // 2, (h % 2) * 64
                    qs = qT[kch][row0:row0+64, t0:t0+QP]
                    pts = []
                    for kc, (koff, klen) in enumerate(KCH):
                        ps_s = psc.tile([128, QP], F32, tag="at_s")
                        nc.tensor.matmul(
                            ps_s[:klen, :],
                            kT[kch][row0:row0+64, t0+koff:t0+koff+klen],
                            qs, start=True, stop=True)
                        pt = pc.tile([128, QP], BF16, tag="at_p")
                        nc.scalar.activation(out=pt[:klen, :],
                                             in_=ps_s[:klen, :], func=AF.Exp)
                        pm = pc.tile([128, QP], BF16, tag="at_pm")
                        nc.vector.tensor_mul(pm[:klen, :], pt[:klen, :],
                                             ebts[(h, kc)][:klen, :])
                        pts.append(pm)
                    po = psd.tile([65, QP], F32, tag="at_o")
                    for kc, (koff, klen) in enumerate(KCH):
                        nc.tensor.matmul(po, vts[kc][:klen, h*65:(h+1)*65],
                                         pts[kc][:klen, :],
                                         start=(kc == 0), stop=(kc == 2))
                    nc.vector.tensor_copy(odT[kch][row0:row0+64, t0:t0+NT],
                                          po[0:64, 0:NT])
                    srow = pc.tile([1, NT], F32, tag="at_sr")
                    nc.scalar.activation(out=srow, in_=po[64:65, 0:NT],
                                         func=AF.Copy)
                    nc.sync.dma_start(out=sums_t[h:h+1, t0:t0+NT], in_=srow)
        peb_cm.__exit__(None, None, None)
        psd_cm.__exit__(None, None, None)
        psc_cm.__exit__(None, None, None)
        pcb_cm.__exit__(None, None, None)
        pc_cm.__exit__(None, None, None)
        rqk_cm.__exit__(None, None, None)
        if DBG:
            for k in range(KD):
                nc.sync.dma_start(out=dbg_od[k*128:(k+1)*128, :], in_=odT[k])
            nc.sync.dma_start(out=dbg_sums[:, :], in_=sums_t[:, :])

        # ====== D: softmax-normalize + inner LN stats + proj + resid ======
        rowsB_cm = tc.tile_pool(name="rowsB", bufs=1, side="right")
        rowsB = rowsB_cm.__enter__()
        muo_row = rowsB.tile([1, TP], F32R, name="muo_row")
        muo_b = rowsB.tile([1, TP], BF16, name="muo_b")
        rso_row = rowsB.tile([1, TP], F32R, name="rso_row")
        mu2_row = rowsB.tile([1, TP], F32R, name="mu2_row")
        rs2_row = rowsB.tile([1, TP], F32R, name="rs2_row")
        roh_cm = tc.tile_pool(name="roh", bufs=1, side="right")
        roh = roh_cm.__enter__()
        ohT = [roh.tile([128, TP], BF16, name=f"ohT{k}", tag=f"ohT{k}")
               for k in range(KD)]
        with tc.tile_pool(name="pdo", bufs=3) as pdo, \
             tc.tile_pool(name="prd", bufs=1) as prd, \
             tc.tile_pool(name="pds", bufs=1) as pds, \
             tc.tile_pool(name="psf", bufs=2, space="PSUM") as psf, \
             tc.tile_pool(name="pso", bufs=2, space="PSUM") as pso:
            recip = pds.tile([16, TP], F32R)
            for (toff, tlen) in TOK_SLICES:
                nc.vector.reciprocal(recip[:, toff:toff+tlen],
                                     sums_t[:, toff:toff+tlen])
            sel_t = [pds.tile([16, 128], F32R, name=f"sel{k}", tag=f"sel{k}")
                     for k in range(KD)]
            for k in range(KD):
                nc.sync.dma_start(out=sel_t[k],
                                  in_=sel_d[k, :, :].bitcast(F32R))
            for (toff, tlen) in TOK_SLICES:
                o_sx = pso.tile([1, 512], F32, tag="o_sx")
                o_sxx = pso.tile([1, 512], F32, tag="o_sxx")
                for k in range(KD):
                    fld = psf.tile([128, 512], F32, tag="d_fld")
                    nc.tensor.matmul(fld[:, :tlen], sel_t[k],
                                     recip[:, toff:toff+tlen],
                                     start=True, stop=True)
                    nc.vector.tensor_mul(ohT[k][:, toff:toff+tlen],
                                         odT[k][:, toff:toff+tlen],
                                         fld[:, :tlen])
                    nc.tensor.matmul(o_sx[:, :tlen], ones_cb,
                                     ohT[k][:, toff:toff+tlen],
                                     start=(k == 0), stop=(k == KD-1))
                    sq = pdo.tile([128, 512], F32R, tag="d_sq")
                    nc.scalar.activation(out=sq[:, :tlen],
                                         in_=ohT[k][:, toff:toff+tlen],
                                         func=AF.Square)
                    nc.tensor.matmul(o_sxx[:, :tlen], ones_c, sq[:, :tlen],
                                     start=(k == 0), stop=(k == KD-1))
                ln_rows(prd, o_sx, o_sxx, muo_row, rso_row, toff, tlen, D,
                        tagp="d_")
                nc.vector.tensor_copy(muo_b[:, toff:toff+tlen],
                                      muo_row[:, toff:toff+tlen])
        rod_cm.__exit__(None, None, None)
        rowsA_cm.__exit__(None, None, None)

        # proj + residual + norm2 stats
        rx2_cm = tc.tile_pool(name="rx2", bufs=1)
        rx2 = rx2_cm.__enter__()
        x2 = [rx2.tile([128, TP], F32R, name=f"x2_{m}", tag=f"x2_{m}")
              for m in range(KD)]
        rof = rx2.tile([128, TP], BF16, name="rof")
        with tc.tile_pool(name="pdp", bufs=3) as pdp, \
             tc.tile_pool(name="prp", bufs=1) as prp, \
             tc.tile_pool(name="pdw", bufs=1) as pdw, \
             tc.tile_pool(name="psp", bufs=2, space="PSUM") as psp, \
             tc.tile_pool(name="pss", bufs=2, space="PSUM") as pss:
            wp = [pdw.tile([128, D], BF16, name=f"wp{k}", tag=f"wp{k}")
                  for k in range(KD)]
            for k in range(KD):
                nc.sync.dma_start(out=wp[k], in_=wpT_d[k*128:(k+1)*128, :])
            cp_t = pdw.tile([1, D], BF16)
            nc.sync.dma_start(out=cp_t, in_=cpn[:, :])
            pb_t = pdw.tile([128, KD], F32)
            nc.sync.dma_start(out=pb_t, in_=pb_c[:, :])
            for (toff, tlen) in TOK_SLICES:
                ps_b = psp.tile([128, 512], F32, tag="rof_bc")
                nc.tensor.matmul(ps_b[:, :tlen], ones_r,
                                 rso_row[:, toff:toff+tlen],
                                 start=True, stop=True)
                nc.scalar.activation(out=rof[:, toff:toff+tlen],
                                     in_=ps_b[:, :tlen], func=AF.Copy)
            for (toff, tlen) in TOK_SLICES:
                x_sx = pss.tile([1, 512], F32, tag="x_sx")
                x_sxx = pss.tile([1, 512], F32, tag="x_sxx")
                for m in range(KD):
                    ps = psp.tile([128, 512], F32, tag="pj_ps")
                    for k in range(KD):
                        nc.tensor.matmul(ps[:, :tlen],
                                         wp[k][:, m*128:(m+1)*128],
                                         ohT[k][:, toff:toff+tlen],
                                         start=(k == 0), stop=False)
                    nc.tensor.matmul(ps[:, :tlen], cp_t[:, m*128:(m+1)*128],
                                     muo_b[:, toff:toff+tlen],
                                     start=False, stop=True)
                    tt = pdp.tile([128, 512], F32R, tag="pj_t")
                    nc.vector.tensor_mul(tt[:, :tlen], ps[:, :tlen],
                                         rof[:, toff:toff+tlen])
                    xs = pdp.tile([128, 512], F32, tag="pj_xs")
                    nc.sync.dma_start(out=xs[:, :tlen],
                                      in_=xT_d[m*128:(m+1)*128,
                                               toff:toff+tlen])
                    nc.vector.scalar_tensor_tensor(
                        out=x2[m][:, toff:toff+tlen], in0=tt[:, :tlen],
                        scalar=pb_t[:, m:m+1], in1=xs[:, :tlen],
                        op0=ALU.add, op1=ALU.add)
                    nc.sync.dma_start(
                        out=x2d[m*128:(m+1)*128, toff:toff+tlen].bitcast(F32R),
                        in_=x2[m][:, toff:toff+tlen])
                    nc.tensor.matmul(x_sx[:, :tlen], ones_c,
                                     x2[m][:, toff:toff+tlen],
                                     start=(m == 0), stop=(m == KD-1))
                    sq = pdp.tile([128, 512], F32R, tag="pj_sq")
                    nc.scalar.activation(out=sq[:, :tlen],
                                         in_=x2[m][:, toff:toff+tlen],
                                         func=AF.Square)
                    nc.tensor.matmul(x_sxx[:, :tlen], ones_c, sq[:, :tlen],
                                     start=(m == 0), stop=(m == KD-1))
                ln_rows(prp, x_sx, x_sxx, mu2_row, rs2_row, toff, tlen, D,
                        tagp="x2_")
            if DBG:
                for m in range(KD):
                    nc.sync.dma_start(
                        out=dbg_x2[m*128:(m+1)*128, :].bitcast(F32R),
                        in_=x2[m])
        roh_cm.__exit__(None, None, None)

        # ================= E0: xhat2 =================
        rx3_cm = tc.tile_pool(name="rx3", bufs=1, side="right")
        rx3 = rx3_cm.__enter__()
        xh2 = [rx3.tile([128, TP], BF16, name=f"xh2_{k}", tag=f"xh2_{k}")
               for k in range(KD)]
        with tc.tile_pool(name="pe0", bufs=3) as pe0, \
             tc.tile_pool(name="pse0", bufs=2, space="PSUM") as pse0:
            for (toff, tlen) in TOK_SLICES:
                ps_mu = pse0.tile([128, 512], F32, tag="mu2_bc")
                nc.tensor.matmul(ps_mu[:, :tlen], ones_r,
                                 mu2_row[:, toff:toff+tlen],
                                 start=True, stop=True)
                mu2f = pe0.tile([128, 512], F32, tag="mu2f")
                nc.scalar.activation(out=mu2f[:, :tlen], in_=ps_mu[:, :tlen],
                                     func=AF.Copy)
                ps_rs = pse0.tile([128, 512], F32, tag="rs2_bc")
                nc.tensor.matmul(ps_rs[:, :tlen], ones_r,
                                 rs2_row[:, toff:toff+tlen],
                                 start=True, stop=True)
                rs2f = pe0.tile([128, 512], BF16, tag="rs2f")
                nc.scalar.activation(out=rs2f[:, :tlen], in_=ps_rs[:, :tlen],
                                     func=AF.Copy)
                for k in range(KD):
                    df = pe0.tile([128, 512], BF16, tag="e0_d")
                    nc.vector.tensor_sub(df[:, :tlen],
                                         x2[k][:, toff:toff+tlen],
                                         mu2f[:, :tlen])
                    nc.vector.tensor_mul(xh2[k][:, toff:toff+tlen],
                                         df[:, :tlen], rs2f[:, :tlen])
            if DBG:
                for k in range(KD):
                    nc.sync.dma_start(out=dbg_xh2[k*128:(k+1)*128, :],
                                      in_=xh2[k])
        rx2_cm.__exit__(None, None, None)

        # ================= E: w1/w2 + silu -> hd =================
        with tc.tile_pool(name="pe", bufs=3, side="right") as pe, \
             tc.tile_pool(name="pew", bufs=2, side="right") as pew, \
             tc.tile_pool(name="pec", bufs=1, side="right") as pec, \
             tc.tile_pool(name="pse1", bufs=3, space="PSUM", side="right") as pse1, \
             tc.tile_pool(name="pse2", bufs=3, space="PSUM", side="right") as pse2:
            w1b_t = pec.tile([128, MH], F32)
            w2b_t = pec.tile([128, MH], F32)
            nc.sync.dma_start(out=w1b_t, in_=w1b_c[:, :])
            nc.sync.dma_start(out=w2b_t, in_=w2b_c[:, :])
            for m in range(MH):
                w1a = pew.tile([128, D], BF16, tag="w1a")
                w2a = pew.tile([128, D], BF16, tag="w2a")
                _we1 = (nc.sync, nc.gpsimd)[m % 2]
                _we2 = (nc.gpsimd, nc.sync)[m % 2]
                _we1.dma_start(
                    out=w1a.rearrange("p (g c) -> p g c", g=KD),
                    in_=w1T_d[:, m*128:(m+1)*128].rearrange(
                        "(g p) c -> p g c", p=128))
                _we2.dma_start(
                    out=w2a.rearrange("p (g c) -> p g c", g=KD),
                    in_=w2T_d[:, m*128:(m+1)*128].rearrange(
                        "(g p) c -> p g c", p=128))
                w1m = [w1a[:, k*128:(k+1)*128] for k in range(KD)]
                w2m = [w2a[:, k*128:(k+1)*128] for k in range(KD)]
                for (toff, tlen) in TOK_SLICES:
                    ps1 = pse1.tile([128, 512], F32, tag="e_ps1")
                    for k in range(KD):
                        nc.tensor.matmul(ps1[:, :tlen], w1m[k],
                                         xh2[k][:, toff:toff+tlen],
                                         start=(k == 0), stop=(k == KD-1))
                    h1 = pe.tile([128, 512], BF16, tag="e_h1")
                    nc.scalar.activation(out=h1[:, :tlen], in_=ps1[:, :tlen],
                                         func=AF.Silu, bias=w1b_t[:, m:m+1],
                                         scale=1.0)
                    ps2 = pse2.tile([128, 512], F32, tag="e_ps2")
                    for k in range(KD):
                        nc.tensor.matmul(ps2[:, :tlen], w2m[k],
                                         xh2[k][:, toff:toff+tlen],
                                         start=(k == 0), stop=(k == KD-1))
                    ht = pe.tile([128, 512], BF16, tag="e_ht")
                    nc.vector.scalar_tensor_tensor(
                        out=ht[:, :tlen], in0=ps2[:, :tlen],
                        scalar=w2b_t[:, m:m+1], in1=h1[:, :tlen],
                        op0=ALU.add, op1=ALU.mult)
                    nc.gpsimd.dma_start(out=hd[m*128:(m+1)*128,
                                                toff:toff+tlen],
                                         in_=ht[:, :tlen])
        rx3_cm.__exit__(None, None, None)
        rowsB_cm.__exit__(None, None, None)

        # ============ G: ffn-LN stats (JIT) + w3 + resid -> outT ============
        with tc.tile_pool(name="pg", bufs=3) as pg, \
             tc.tile_pool(name="prg", bufs=1) as prg, \
             tc.tile_pool(name="pgh", bufs=2) as pgh, \
             tc.tile_pool(name="pgw", bufs=1) as pgw, \
             tc.tile_pool(name="psg", bufs=1, space="PSUM") as psg, \
             tc.tile_pool(name="psh", bufs=1, space="PSUM") as psh:
            w3 = [pgw.tile([128, D], BF16, name=f"w3_{mh}", tag=f"w3_{mh}")
                  for mh in range(MH)]
            for mh in range(MH):
                nc.sync.dma_start(out=w3[mh], in_=w3T_d[mh*128:(mh+1)*128, :])
            c3_t = pgw.tile([1, D], BF16)
            nc.sync.dma_start(out=c3_t, in_=c3n[:, :])
            w3b_t = pgw.tile([128, KD], F32)
            nc.sync.dma_start(out=w3b_t, in_=w3b_c[:, :])
            for (toff, tlen) in TOK_SLICES:
                hts = []
                for mh in range(MH):
                    htl = pgh.tile([128, 512], BF16, tag=f"g_h{mh}")
                    nc.sync.dma_start(out=htl[:, :tlen],
                                      in_=hd[mh*128:(mh+1)*128,
                                             toff:toff+tlen])
                    hts.append(htl)
                h_sx = psh.tile([1, 512], F32, tag="h_sx")
                h_sxx = psh.tile([1, 512], F32, tag="h_sxx")
                psA = [psg.tile([128, 512], F32, name=f"g_psA{m}", tag=f"g_ps{m}")
                       for m in range(4)]
                for mh in range(MH):
                    for m in range(4):
                        nc.tensor.matmul(psA[m][:, :tlen], w3[mh][:, m*128:(m+1)*128],
                                         hts[mh][:, :tlen],
                                         start=(mh == 0), stop=False)
                    nc.tensor.matmul(h_sx[:, :tlen], ones_cb, hts[mh][:, :tlen],
                                     start=(mh == 0), stop=(mh == MH-1))
                    sqh = pg.tile([128, 512], F32R, tag="g_sq")
                    nc.scalar.activation(out=sqh[:, :tlen],
                                         in_=hts[mh][:, :tlen], func=AF.Square)
                    nc.tensor.matmul(h_sxx[:, :tlen], ones_c, sqh[:, :tlen],
                                     start=(mh == 0), stop=(mh == MH-1))
                muh_sl = prg.tile([1, 512], F32R, tag="g_muh")
                rsh_sl = prg.tile([1, 512], F32R, tag="g_rsh")
                ln_rows(prg, h_sx, h_sxx, muh_sl, rsh_sl, 0, tlen, HID,
                        tagp="g_")
                muh_b = prg.tile([1, 512], BF16, tag="g_muhb")
                nc.vector.tensor_copy(muh_b[:, :tlen], muh_sl[:, :tlen])
                ps_rh = psh.tile([128, 512], F32, tag="rh_bc")
                nc.tensor.matmul(ps_rh[:, :tlen], ones_r, rsh_sl[:, :tlen],
                                 start=True, stop=True)
                rhf = pg.tile([128, 512], BF16, tag="g_rhf")
                nc.scalar.activation(out=rhf[:, :tlen], in_=ps_rh[:, :tlen],
                                     func=AF.Copy)

                def drain(m, psm):
                    nc.tensor.matmul(psm[:, :tlen], c3_t[:, m*128:(m+1)*128],
                                     muh_b[:, :tlen], start=False, stop=True)
                    tt = pg.tile([128, 512], F32R, tag="g_t")
                    nc.vector.tensor_mul(tt[:, :tlen], psm[:, :tlen],
                                         rhf[:, :tlen])
                    x2s = pg.tile([128, 512], F32, tag="g_x2")
                    nc.sync.dma_start(out=x2s[:, :tlen],
                                      in_=x2d[m*128:(m+1)*128, toff:toff+tlen])
                    ot = pg.tile([128, 512], F32R, tag="g_ot")
                    nc.vector.scalar_tensor_tensor(
                        out=ot[:, :tlen], in0=tt[:, :tlen],
                        scalar=w3b_t[:, m:m+1], in1=x2s[:, :tlen],
                        op0=ALU.add, op1=ALU.add)
                    nc.sync.dma_start(
                        out=out[m*128:(m+1)*128, toff:toff+tlen].bitcast(F32R),
                        in_=ot[:, :tlen])

                for m in range(4):
                    drain(m, psA[m])
                psB = [psg.tile([128, 512], F32, name=f"g_psB{m}", tag=f"g_ps{m}")
                       for m in range(4)]
                for mh in range(MH):
                    for m in range(4):
                        nc.tensor.matmul(psB[m][:, :tlen],
                                         w3[mh][:, (m+4)*128:(m+5)*128],
                                         hts[mh][:, :tlen],
                                         start=(mh == 0), stop=False)
                for m in range(4):
                    drain(m + 4, psB[m])

        consts_cm.__exit__(None, None, None)
    _fix_wait_limits(nc, mybir)
    _CACHE[key] = nc
    return nc


def prep_inputs(inputs):
    f32 = np.float32
    import ml_dtypes
    bf16 = ml_dtypes.bfloat16
    x = np.asarray(inputs['x'], f32)
    g1 = np.asarray(inputs['norm1_g'], f32); b1 = np.asarray(inputs['norm1_b'], f32)
    gi = np.asarray(inputs['inner_ln_g'], f32); bi = np.asarray(inputs['inner_ln_b'], f32)
    g2 = np.asarray(inputs['norm2_g'], f32); b2 = np.asarray(inputs['norm2_b'], f32)
    gf = np.asarray(inputs['ffn_ln_g'], f32); bf = np.asarray(inputs['ffn_ln_b'], f32)
    q_w = np.asarray(inputs['q_w'], f32); q_b = np.asarray(inputs['q_b'], f32)
    k_w = np.asarray(inputs['k_w'], f32)
    v_w = np.asarray(inputs['v_w'], f32); v_b = np.asarray(inputs['v_b'], f32)
    p_w = np.asarray(inputs['proj_w'], f32); p_b = np.asarray(inputs['proj_b'], f32)
    w1_w = np.asarray(inputs['w1_w'], f32); w1_b = np.asarray(inputs['w1_b'], f32)
    w2_w = np.asarray(inputs['w2_w'], f32); w2_b = np.asarray(inputs['w2_b'], f32)
    w3_w = np.asarray(inputs['w3_w'], f32); w3_b = np.asarray(inputs['w3_b'], f32)
    tab = np.asarray(inputs['rel_bias_table'], f32)
    rc = np.asarray(inputs['rope_cos'], f32)
    rs = np.asarray(inputs['rope_sin'], f32)

    wqTe = (q_w.T * g1[:, None] * SCALE).astype(f32)
    qb_eff = ((q_b + q_w @ b1) * SCALE).astype(f32)
    wkTe = (k_w.T * g1[:, None]).astype(f32)
    kb_eff = (k_w @ b1).astype(f32)
    wvTe = (v_w.T * g1[:, None]).astype(f32)
    vb_eff = (v_b + v_w @ b1).astype(f32)
    wpTe = (p_w.T * gi[:, None]).astype(f32)
    pb_eff = (p_b + p_w @ bi).astype(f32)
    w1Te = (w1_w.T * g2[:, None]).astype(f32)
    w1b_eff = (w1_b + w1_w @ b2).astype(f32)
    w2Te = (w2_w.T * g2[:, None]).astype(f32)
    w2b_eff = (w2_b + w2_w @ b2).astype(f32)
    w3Te = (w3_w.T * gf[:, None]).astype(f32)
    w3b_eff = (w3_b + w3_w @ bf).astype(f32)

    cosr = np.ones((128, TP), f32)
    sinr = np.zeros((128, TP), f32)
    for b in range(BL):
        cosr[0:64, b*NT+1:(b+1)*NT] = rc.T
        cosr[64:128, b*NT+1:(b+1)*NT] = rc.T
        sinr[0:64, b*NT+1:(b+1)*NT] = rs.T
        sinr[64:128, b*NT+1:(b+1)*NT] = rs.T

    r2 = np.zeros((64, 64), f32)
    for i in range(32):
        r2[2*i, 2*i+1] = -1.0
        r2[2*i+1, 2*i] = 1.0
    r2b = np.zeros((128, 128), f32)
    r2b[0:64, 0:64] = r2
    r2b[64:128, 64:128] = r2

    idx = _rel_pos_index()
    bias_full = tab[idx, :]                      # [NT(q), NT(k), H]
    expb = np.zeros((H, 3, 128, QP), f32)
    for h in range(H):
        bh = np.exp(bias_full[:, :, h].T)        # [key, query]
        for kc, (koff, klen) in enumerate(KCH):
            expb[h, kc, :klen, :NT] = bh[koff:koff+klen, :]
    expb[:, :, :, NT:] = 1.0

    sel = np.zeros((KD, 16, 128), f32)
    for k in range(KD):
        sel[k, 2*k, 0:64] = 1.0
        sel[k, 2*k+1, 64:128] = 1.0

    maskr = np.zeros((1, TP), f32)
    maskr[0, :T] = 1.0

    common = {
        'wqT': wqTe, 'wkT': wkTe, 'wvT': wvTe,
        'wpT': wpTe.astype(bf16), 'w1T': w1Te.astype(bf16),
        'w2T': w2Te.astype(bf16), 'w3T': w3Te.astype(bf16),
        'cqn': -wqTe.sum(0, keepdims=True), 'ckn': -wkTe.sum(0, keepdims=True),
        'cvn': -wvTe.sum(0, keepdims=True),
        'cpn': (-wpTe.astype(bf16).astype(f32).sum(0, keepdims=True)).astype(bf16),
        'c3n': (-w3Te.astype(bf16).astype(f32).sum(0, keepdims=True)).astype(bf16),
        'qb_c': qb_eff.reshape(KD, 128).T.copy(),
        'kb_c': kb_eff.reshape(KD, 128).T.copy(),
        'pb_c': pb_eff.reshape(KD, 128).T.copy(),
        'w1b_c': w1b_eff.reshape(MH, 128).T.copy(),
        'w2b_c': w2b_eff.reshape(MH, 128).T.copy(),
        'w3b_c': w3b_eff.reshape(KD, 128).T.copy(),
        'vbr': vb_eff.reshape(1, D).astype(bf16),
        'cosr': cosr.astype(bf16), 'sinr': sinr.astype(bf16),
        'r2T': r2b.T.copy().astype(bf16),
        'expb': expb.astype(bf16), 'sel': sel,
        'onesc': np.ones((128, 1), f32), 'onesr': np.ones((1, 128), f32),
        'onespat': np.ones((128, 16), f32).astype(bf16),
        'maskr': maskr,
    }
    in_maps = []
    for c in range(NCORES):
        xc = np.zeros((TP, D), f32)
        xc[:T] = x[c*BL:(c+1)*BL].reshape(T, D)
        im = dict(common)
        im['xT'] = np.ascontiguousarray(xc.T)
        in_maps.append(im)
    return in_maps


def postprocess_out(full):
    """full: concatenated per-core 'out' arrays, shape (8*D, TP)."""
    full = np.asarray(full).reshape(NCORES, D, TP)
    outs = [full[c].T[:T].reshape(BL, NT, D) for c in range(NCORES)]
    return np.concatenate(outs, 0).astype(np.float32)


def kernel(**inputs):
    from concourse.bass_utils import run_bass_kernel_spmd
    nc = build_module()
    in_maps = prep_inputs(inputs)
    res = run_bass_kernel_spmd(nc, in_maps, list(range(NCORES)))
    outs = []
    for c in range(NCORES):
        outs.append(res.results[c]['out'].T[:T].reshape(BL, NT, D))
    return np.concatenate(outs, 0).astype(np.float32)



# revision 2
# speedup vs baseline: 1.0964x; 1.0964x over previous
"""Trainium2 Bass kernel v2 for nn_Block_24532853194876.

Feature-major design:
  - x uploaded pre-transposed [D, TP]; LN stats via ACT-square + PE
    ones-matmul reductions; mean removal folded into matmuls as rank-1
    accumulation steps; rstd folded into rope fields (q/k), per-token
    STT scale (v), and broadcast fields (proj/w3 outputs).
  - q/k/od kept SBUF-resident in bf16; v via DRAM in a per-head
    65-col-stride layout with a built-in ones column for softmax sums.
  - softmax: exp on ACT (PSUM->bf16), exp(bias) multiplied in bf16,
    normalization deferred to phase D via a reciprocal field built with
    selector matmuls.
  - MLP in bf16; ffn-LN stats computed in G per token-slice just in time.
Output is feature-major outT [D, TP]; host transposes.
"""
import sys
sys.path.insert(0, '/opt/trn_rl_repo')
import numpy as np

B, NT, D, H, HD, HID = 64, 257, 1024, 16, 64, 4096
NCORES = 8
BL = B // NCORES
T = BL * NT                 # 2056
TP = 2176                   # 17*128
NCH = TP // 128             # 17
KD = D // 128               # 8
MH = HID // 128             # 32
QP = 260
LN_EPS = 1e-5
WS = 16
SCALE = HD ** -0.5
TOK_SLICES = [(0, 412), (412, 412), (824, 412), (1236, 412), (1648, 408)]
VCH = [(i * 128, 128) for i in range(16)] + [(2048, 8)]
KCH = [(0, 128), (128, 128), (256, 1)]
VROW = H * 65               # 1040

_CACHE = {}


def _rel_pos_index():
    ch, cw = np.meshgrid(np.arange(WS), np.arange(WS), indexing='ij')
    flat = np.stack([ch.reshape(-1), cw.reshape(-1)])
    rel = flat[:, :, None] - flat[:, None, :]
    rel = rel.transpose(1, 2, 0).astype(np.int64)
    rel[:, :, 0] += WS - 1
    rel[:, :, 1] += WS - 1
    rel[:, :, 0] *= 2 * WS - 1
    nrd = (2 * WS - 1) * (2 * WS - 1) + 3
    idx = np.zeros((NT, NT), dtype=np.int64)
    idx[1:, 1:] = rel.sum(-1)
    idx[0, 0:] = nrd - 3
    idx[0:, 0] = nrd - 2
    idx[0, 0] = nrd - 1
    return idx


def _fix_wait_limits(nc, mybir, max_waits=1):
    for f in nc.m.functions:
        for bb in f.blocks:
            insts = bb.instructions
            i = 0
            while i < len(insts):
                inst = insts[i]
                si = inst.sync_info
                if si and si.on_wait and len(si.on_wait) > max_waits:
                    extra = si.on_wait[:-max_waits]
                    inst.sync_info.on_wait = si.on_wait[-max_waits:]
                    for j, w in enumerate(extra):
                        nop = mybir.InstNoOp(
                            name=f"{inst.name}-ws{j}", engine=inst.engine,
                            ins=[], outs=[],
                            sync_info=mybir.SyncInfo(on_wait=[w], on_update=[]),
                        )
                        insts.insert(i, nop)
                        i += 1
                i += 1


def build_module():
    import os
    DBG = os.environ.get('KDBG') == '1'
    REP = int(os.environ.get('KREPEAT') or '1')
    key = ('v2', DBG, REP)
    if key in _CACHE:
        return _CACHE[key]
    import concourse.bass as bass
    import concourse.mybir as mybir
    import concourse.tile as tile

    F32 = mybir.dt.float32
    F32R = mybir.dt.float32r
    BF16 = mybir.dt.bfloat16
    AF = mybir.ActivationFunctionType
    ALU = mybir.AluOpType

    nc = bass.Bass()
    P = lambda name, shape, dt=F32: nc.declare_dram_parameter(
        name, shape, dt, isOutput=False)

    xT_d = P("xT", [D, TP])                 # raw x transposed, pad zeros
    wqT = P("wqT", [D, D])                  # [in, out] effective (g1, scale)
    wkT = P("wkT", [D, D])
    wvT = P("wvT", [D, D])
    wpT_d = P("wpT", [D, D], BF16)
    w1T_d = P("w1T", [D, HID], BF16)
    w2T_d = P("w2T", [D, HID], BF16)
    w3T_d = P("w3T", [HID, D], BF16)
    cpn = P("cpn", [1, D], BF16)
    c3n = P("c3n", [1, D], BF16)
    qb_c = P("qb_c", [128, KD])
    kb_c = P("kb_c", [128, KD])
    pb_c = P("pb_c", [128, KD])
    w1b_c = P("w1b_c", [128, MH])
    w2b_c = P("w2b_c", [128, MH])
    w3b_c = P("w3b_c", [128, KD])
    vbr = P("vbr", [1, D], BF16)
    cosr = P("cosr", [128, TP], BF16)
    sinr = P("sinr", [128, TP], BF16)
    r2T_d = P("r2T", [128, 128], BF16)
    expb2_d = P("expb2", [128, (H // 2) * 2 * 2 * QP], BF16)
    expbc_d = P("expbc", [65, (H // 2) * QP], BF16)
    sel_d = P("sel", [KD, 16, 128])
    onesc_d = P("onesc", [128, 1])
    onesr_d = P("onesr", [1, 128])
    onespat_d = P("onespat", [128, 16], BF16)
    mask_d = P("maskr", [1, TP])
    out = nc.declare_dram_parameter("out", [D, TP], F32, isOutput=True)

    if DBG:
        S = lambda name, shape, dt=F32: nc.declare_dram_parameter(
            name, shape, dt, isOutput=True)
        dbg_q = S("dbg_q", [D, TP], BF16)
        dbg_od = S("dbg_od", [D, TP], F32)
        dbg_sums = S("dbg_sums", [16, TP])
        dbg_x2 = S("dbg_x2", [D, TP])
        dbg_mu = S("dbg_mu", [1, TP])
        dbg_rs = S("dbg_rs", [1, TP])
        dbg_xh2 = S("dbg_xh2", [D, TP], BF16)
    vd = nc.dram_tensor("vd", [TP, VROW], BF16)
    rs_stage = nc.dram_tensor("rs_stage", [1, TP], F32)
    hd = nc.dram_tensor("hd", [MH // 4, 128, 4, TP], BF16)
    x2d = nc.dram_tensor("x2d", [D, TP], F32)

    with tile.TileContext(nc) as tc, \
         nc.allow_low_precision(reason="intentional f32r/bf16 pipeline"):
      for _rep in range(REP):
        consts_cm = tc.tile_pool(name="consts", bufs=1)
        consts = consts_cm.__enter__()
        ones_c = consts.tile([128, 1], F32R)
        nc.sync.dma_start(out=ones_c, in_=onesc_d[:, :].bitcast(F32R))
        ones_cb = consts.tile([128, 1], BF16)
        onesf_t = consts.tile([128, 1], F32)
        nc.vector.memset(onesf_t, 1.0)
        nc.vector.tensor_copy(ones_cb, onesf_t)
        ones_r = consts.tile([1, 128], F32R)
        nc.sync.dma_start(out=ones_r, in_=onesr_d[:, :].bitcast(F32R))
        eps1_t = consts.tile([1, 1], F32)
        nc.vector.memset(eps1_t, LN_EPS)

        rowsA_cm = tc.tile_pool(name="rowsA", bufs=1)
        rowsA = rowsA_cm.__enter__()
        murow = rowsA.tile([1, TP], F32R, name="murow")
        rstok = rowsA.tile([128, NCH], F32, name="rstok")
        sums_t = rowsA.tile([16, TP], F32, name="sums_t")
        rqk_cm = tc.tile_pool(name="rqk", bufs=1, side="right")
        rqk = rqk_cm.__enter__()
        qT = [rqk.tile([128, TP], BF16, name=f"qT{k}", tag=f"qT{k}")
              for k in range(KD)]
        kT = [rqk.tile([128, TP], BF16, name=f"kT{k}", tag=f"kT{k}")
              for k in range(KD)]
        rfld_cm = tc.tile_pool(name="rfld", bufs=1, side="right")
        rfld = rfld_cm.__enter__()
        rsrow = rfld.tile([1, TP], F32R, name="rsrow")
        cosf = rfld.tile([128, TP], BF16, name="cosf")
        sinf = rfld.tile([128, TP], BF16, name="sinf")

        def ln_rows(pool, ps_sx, ps_sxx, mu_dst, rs_dst, toff, tlen, ndim,
                    mask=None, tagp="", bufs_note=None):
            """mu/rstd rows for one token slice from sum/sumsq psum tiles."""
            nc.scalar.activation(out=mu_dst[:, toff:toff+tlen],
                                 in_=ps_sx[:, :tlen], func=AF.Copy,
                                 scale=1.0/ndim)
            ex2 = pool.tile([1, 512], F32, tag=tagp+"ex2")
            nc.scalar.activation(out=ex2[:, :tlen], in_=ps_sxx[:, :tlen],
                                 func=AF.Copy, scale=1.0/ndim)
            mu2 = pool.tile([1, 512], F32, tag=tagp+"mu2")
            nc.vector.tensor_mul(mu2[:, :tlen], mu_dst[:, toff:toff+tlen],
                                 mu_dst[:, toff:toff+tlen])
            var = pool.tile([1, 512], F32, tag=tagp+"var")
            nc.vector.tensor_sub(var[:, :tlen], ex2[:, :tlen], mu2[:, :tlen])
            sd = pool.tile([1, 512], F32, tag=tagp+"sd")
            nc.scalar.activation(out=sd[:, :tlen], in_=var[:, :tlen],
                                 func=AF.Sqrt, bias=eps1_t[:, 0:1], scale=1.0)
            if mask is None:
                nc.vector.reciprocal(rs_dst[:, toff:toff+tlen], sd[:, :tlen])
            else:
                rcp = pool.tile([1, 512], F32, tag=tagp+"rcp")
                nc.vector.reciprocal(rcp[:, :tlen], sd[:, :tlen])
                nc.vector.tensor_mul(rs_dst[:, toff:toff+tlen], rcp[:, :tlen],
                                     mask[:, toff:toff+tlen])

        # ================= A: xT in + norm1 stats + centering =================
        resA_cm = tc.tile_pool(name="resA", bufs=1)
        resA = resA_cm.__enter__()
        xT = [resA.tile([128, TP], F32R, name=f"xT{k}", tag=f"xT{k}")
              for k in range(KD)]
        _qeng = [nc.sync, nc.scalar]
        t1off = TOK_SLICES[0][1]
        for k in range(KD):
            _qeng[k % 2].dma_start(
                out=xT[k][:, 0:t1off],
                in_=xT_d[k*128:(k+1)*128, 0:t1off].bitcast(F32R))
        for k in range(KD):
            _qeng[k % 2].dma_start(
                out=xT[k][:, t1off:TP],
                in_=xT_d[k*128:(k+1)*128, t1off:TP].bitcast(F32R))
        with tc.tile_pool(name="pa", bufs=3) as pa, \
             tc.tile_pool(name="pra", bufs=1) as pra, \
             tc.tile_pool(name="psa", bufs=2, space="PSUM") as psa, \
             tc.tile_pool(name="psamu", bufs=2, space="PSUM") as psamu:
            for (toff, tlen) in TOK_SLICES:
                ps_sx = psa.tile([1, 512], F32, tag="a_sx")
                ps_sxx = psa.tile([1, 512], F32, tag="a_sxx")
                for k in range(KD):
                    nc.tensor.matmul(ps_sx[:, :tlen], ones_c,
                                     xT[k][:, toff:toff+tlen],
                                     start=(k == 0), stop=(k == KD-1))
                    sq = pa.tile([128, 512], F32R, tag="a_sq")
                    xs_ = xT[k][:, toff:toff+tlen]
                    if k % 2 == 0:
                        nc.scalar.activation(out=sq[:, :tlen], in_=xs_,
                                             func=AF.Square)
                    else:
                        nc.vector.tensor_mul(sq[:, :tlen], xs_, xs_)
                    nc.tensor.matmul(ps_sxx[:, :tlen], ones_c, sq[:, :tlen],
                                     start=(k == 0), stop=(k == KD-1))
                ln_rows(pra, ps_sx, ps_sxx, murow, rsrow, toff, tlen, D,
                        tagp="a_")
                # center x in place: x -= mu (broadcast field via PE)
                ps_mu = psamu.tile([128, 512], F32, tag="a_mubc")
                nc.tensor.matmul(ps_mu[:, :tlen], ones_r,
                                 murow[:, toff:toff+tlen],
                                 start=True, stop=True)
                muf = pa.tile([128, 512], F32, tag="a_muf")
                nc.scalar.activation(out=muf[:, :tlen], in_=ps_mu[:, :tlen],
                                     func=AF.Copy)
                for k in range(KD):
                    xs_ = xT[k][:, toff:toff+tlen]
                    nc.vector.tensor_sub(xs_, xs_, muf[:, :tlen])
            if DBG:
                nc.sync.dma_start(out=dbg_mu[:, :], in_=murow[:, :].bitcast(F32))
                nc.sync.dma_start(out=dbg_rs[:, :], in_=rsrow[:, :].bitcast(F32))
            # per-token rstd columns (for v): bounce via DRAM, transposed read
            nc.sync.dma_start(out=rs_stage[0:1, :],
                              in_=rsrow[0:1, :].bitcast(F32))
            nc.sync.dma_start(
                out=rstok,
                in_=rs_stage[0:1, :].rearrange("o (c p) -> (o p) c", p=128))

        # ============ rope fields cos'=cos*rstd, sin'=sin*rstd ============
        with tc.tile_pool(name="psb0", bufs=2, space="PSUM") as psb0:
            nc.sync.dma_start(out=cosf, in_=cosr[:, :])
            nc.scalar.dma_start(out=sinf, in_=sinr[:, :])
            for (toff, tlen) in TOK_SLICES:
                ps_b = psb0.tile([128, 512], F32, tag="rs_bc")
                nc.tensor.matmul(ps_b[:, :tlen], ones_r,
                                 rsrow[:, toff:toff+tlen], start=True, stop=True)
                nc.vector.tensor_mul(cosf[:, toff:toff+tlen],
                                     cosf[:, toff:toff+tlen], ps_b[:, :tlen])
                nc.vector.tensor_mul(sinf[:, toff:toff+tlen],
                                     sinf[:, toff:toff+tlen], ps_b[:, :tlen])

        # ================= B1: q, k (feature-major, rope) =================
        with tc.tile_pool(name="pb1", bufs=3) as pb1, \
             tc.tile_pool(name="pw1", bufs=2) as pw1, \
             tc.tile_pool(name="pwc", bufs=1) as pwc, \
             tc.tile_pool(name="psb1", bufs=4, space="PSUM") as psb1, \
             tc.tile_pool(name="psb2", bufs=3, space="PSUM") as psb2:
            r2t = pwc.tile([128, 128], BF16)
            nc.sync.dma_start(out=r2t, in_=r2T_d[:, :])
            qb_t = pwc.tile([128, KD], F32)
            kb_t = pwc.tile([128, KD], F32)
            nc.sync.dma_start(out=qb_t, in_=qb_c[:, :])
            nc.sync.dma_start(out=kb_t, in_=kb_c[:, :])
            for (wsrc, bcol, dst) in (
                    (wqT, qb_t, qT), (wkT, kb_t, kT)):
                for m in range(KD):
                    wma = pw1.tile([128, D], F32R, tag="wma")
                    _we = (nc.sync, nc.gpsimd, nc.scalar)[m % 3]
                    _we.dma_start(
                        out=wma.rearrange("p (g c) -> p g c", g=KD),
                        in_=wsrc[:, m*128:(m+1)*128].bitcast(F32R).rearrange(
                            "(g p) c -> p g c", p=128))
                    wm = [wma[:, k*128:(k+1)*128] for k in range(KD)]
                    for (toff, tlen) in TOK_SLICES:
                        ps = psb1.tile([128, 512], F32, tag="qk_ps")
                        for k in range(KD):
                            nc.tensor.matmul(ps[:, :tlen], wm[k],
                                             xT[k][:, toff:toff+tlen],
                                             start=(k == 0), stop=(k == KD-1))
                        qtmp = pb1.tile([128, 512], BF16, tag="qk_qtmp")
                        nc.scalar.activation(out=qtmp[:, :tlen],
                                             in_=ps[:, :tlen], func=AF.Identity,
                                             bias=bcol[:, m:m+1], scale=1.0)
                        ps2 = psb2.tile([128, 512], F32, tag="qk_ps2")
                        nc.tensor.matmul(ps2[:, :tlen], r2t, qtmp[:, :tlen],
                                         start=True, stop=True)
                        t1 = pb1.tile([128, 512], BF16, tag="qk_t1")
                        nc.vector.tensor_mul(t1[:, :tlen], ps2[:, :tlen],
                                             sinf[:, toff:toff+tlen])
                        t2 = pb1.tile([128, 512], BF16, tag="qk_t2")
                        nc.vector.tensor_mul(t2[:, :tlen], qtmp[:, :tlen],
                                             cosf[:, toff:toff+tlen])
                        nc.vector.tensor_add(dst[m][:, toff:toff+tlen],
                                             t1[:, :tlen], t2[:, :tlen])
            if DBG:
                for k in range(KD):
                    nc.sync.dma_start(out=dbg_q[k*128:(k+1)*128, :], in_=qT[k])
        rfld_cm.__exit__(None, None, None)

        # ====== C sbuf pools open early (right side; overlap with B2) ======
        pc_cm = tc.tile_pool(name="pc", bufs=4, side="right")
        pc = pc_cm.__enter__()
        pcb_cm = tc.tile_pool(name="pcb", bufs=2, side="right")
        pcb = pcb_cm.__enter__()

        # ================= B2: v -> vd DRAM =================
        with tc.tile_pool(name="pb2", bufs=3) as pb2, \
             tc.tile_pool(name="pw2", bufs=1) as pw2, \
             tc.tile_pool(name="pw2h", bufs=1) as pw2h, \
             tc.tile_pool(name="psv", bufs=3, space="PSUM") as psv:

            vb_b = pw2.tile([128, D], BF16)
            nc.sync.dma_start(out=vb_b, in_=vbr[0:1, :].partition_broadcast(128))
            onespat_t = pw2.tile([128, 16], BF16)
            nc.sync.dma_start(out=onespat_t, in_=onespat_d[:, :])
            for ns in range(2):
                wva = pw2h.tile([128, KD * 512], F32R, tag="wva")
                for k in range(KD):
                    _qeng[k % 2].dma_start(
                        out=wva[:, k*512:(k+1)*512],
                        in_=wvT[k*128:(k+1)*128,
                                ns*512:(ns+1)*512].bitcast(F32R))
                wv = [wva[:, k*512:(k+1)*512] for k in range(KD)]
                for c in range(NCH):
                    ps = psv.tile([128, 512], F32, tag="v_ps")
                    for k in range(KD):
                        nc.tensor.matmul(ps, xT[k][:, c*128:(c+1)*128],
                                         wv[k],
                                         start=(k == 0), stop=(k == KD-1))
                    vs = pb2.tile([128, 520], BF16, tag="v_t")
                    nc.vector.tensor_copy(
                        vs.rearrange("p (g c) -> p g c", g=8)[:, :, 64:65],
                        onespat_t[:, ns*8:(ns+1)*8].unsqueeze(-1))
                    nc.vector.scalar_tensor_tensor(
                        out=vs.rearrange("p (g c) -> p g c", g=8)[:, :, 0:64],
                        in0=ps.rearrange("p (g c) -> p g c", g=8),
                        scalar=rstok[:, c:c+1],
                        in1=vb_b[:, ns*512:(ns+1)*512].rearrange(
                            "p (g c) -> p g c", g=8),
                        op0=ALU.mult, op1=ALU.add)
                    nc.scalar.dma_start(
                        out=vd[c*128:(c+1)*128, ns*520:(ns+1)*520], in_=vs)
        resA_cm.__exit__(None, None, None)

        # ================= C: attention (head-pair batched) =================
        # scores for head pair (2p, 2p+1) land in a 2-bank psum super-tile
        # (bank-aligned halves) -> ONE exp + ONE bias-mult per key chunk.
        # key-256 rows for 4 heads share one bank at partition offsets
        # 0/32/64/96. softmax sums ride psum row 64 of the av output and go
        # straight to sums_t via DMA; od rows go to odT (f32) via DMA.
        psc_cm = tc.tile_pool(name="psc", bufs=2, space="PSUM", side="right")
        psc = psc_cm.__enter__()
        psc2_cm = tc.tile_pool(name="psc2", bufs=2, space="PSUM", side="right")
        psc2 = psc2_cm.__enter__()
        psd_cm = tc.tile_pool(name="psd", bufs=2, space="PSUM", side="right")
        psd = psd_cm.__enter__()
        rod_cm = tc.tile_pool(name="rod", bufs=1)
        rod = rod_cm.__enter__()
        odT = [rod.tile([128, TP], BF16, name=f"odT{k}", tag=f"odT{k}")
               for k in range(KD)]
        peb_cm = tc.tile_pool(name="peb", bufs=1)
        peb = peb_cm.__enter__()
        if True:
            nc.vector.memset(sums_t, 1.0)
            for k in range(KD):
                nc.vector.memset(odT[k][:, T:TP], 0.0)
            eball = peb.tile([128, H // 2, 2, 2, QP], BF16, name="eball")
            nc.sync.dma_start(out=eball.rearrange("p a b c d -> p (a b c d)"),
                              in_=expb2_d[:, :])
            ebcall = peb.tile([65, H // 2, QP], BF16, name="ebcall")
            nc.scalar.dma_start(out=ebcall.rearrange("p a b -> p (a b)"),
                                in_=expbc_d[:, :])
            ebp = {(p, kc): eball[:, p, kc, :, :]
                   for p in range(H // 2) for kc in range(2)}
            ebc = [ebcall[:, p, :] for p in range(H // 2)]
            for b in range(BL):
                t0 = b * NT
                vth = {}
                vcl = pcb.tile([128, VROW], BF16, tag="at_vc")
                for ns in range(2):
                    cs = ns * 520
                    v0 = pcb.tile([128, 520], BF16, tag=f"at_v0{ns}")
                    nc.sync.dma_start(out=v0, in_=vd[t0:t0+128, cs:cs+520])
                    v1 = pcb.tile([128, 520], BF16, tag=f"at_v1{ns}")
                    nc.sync.dma_start(out=v1,
                                      in_=vd[t0+128:t0+256, cs:cs+520])
                    vth[(0, ns)] = v0
                    vth[(1, ns)] = v1
                    for j in range(2):
                        nc.sync.dma_start(out=vcl[64*j:64*j+1, cs:cs+520],
                                          in_=vd[t0+256:t0+257, cs:cs+520])
                for p in range(H // 2):
                    cls_ps = psc2.tile([128, 512], F32, tag="at_cls")
                    for j in range(2):
                        row0 = j * 64
                        nc.tensor.matmul(
                            cls_ps[row0:row0+1, 0:QP],
                            kT[p][row0:row0+64, t0+256:t0+257],
                            qT[p][row0:row0+64, t0:t0+QP],
                            start=True, stop=True)
                    pmc = pc.tile([65, QP], BF16, tag="at_pmc")
                    nc.scalar.activation(out=pmc[0:65, :],
                                         in_=cls_ps[0:65, 0:QP],
                                         func=AF.Exp)
                    nc.vector.tensor_mul(pmc[0:65, :], pmc[0:65, :],
                                         ebc[p][0:65, :])
                    pms = []
                    for kc in range(2):
                        koff = kc * 128
                        sc = psc.tile([128, 1024], F32, tag="at_sc")
                        scv = sc.rearrange("p (g c) -> p g c", g=2)
                        for j in range(2):
                            row0 = j * 64
                            nc.tensor.matmul(
                                scv[:, j, 0:QP],
                                kT[p][row0:row0+64, t0+koff:t0+koff+128],
                                qT[p][row0:row0+64, t0:t0+QP],
                                start=True, stop=True)
                        pm = pc.tile([128, 2, QP], BF16, tag="at_pm")
                        nc.scalar.activation(out=pm, in_=scv[:, :, 0:QP],
                                             func=AF.Exp)
                        nc.vector.tensor_mul(pm, pm, ebp[(p, kc)])
                        pms.append(pm)
                    for j in range(2):
                        h = 2 * p + j
                        row0 = j * 64
                        ns = h // 8
                        hc = (h % 8) * 65
                        po = psd.tile([65, QP], F32, tag="at_o")
                        nc.tensor.matmul(po, vth[(0, ns)][:, hc:hc+65],
                                         pms[0][:, j, :],
                                         start=True, stop=False)
                        nc.tensor.matmul(po, vth[(1, ns)][:, hc:hc+65],
                                         pms[1][:, j, :],
                                         start=False, stop=False)
                        nc.tensor.matmul(po, vcl[64*j:64*j+1,
                                                 ns*520+hc:ns*520+hc+65],
                                         pmc[64*j:64*j+1, :],
                                         start=False, stop=True)
                        if j == 0:
                            nc.vector.tensor_copy(
                                odT[p][row0:row0+64, t0:t0+NT],
                                po[0:64, 0:NT])
                        else:
                            nc.scalar.activation(
                                out=odT[p][row0:row0+64, t0:t0+NT],
                                in_=po[0:64, 0:NT], func=AF.Copy)
                        srow = pc.tile([1, NT], F32, tag="at_sr")
                        nc.vector.tensor_copy(srow, po[64:65, 0:NT])
                        nc.sync.dma_start(
                            out=sums_t[h:h+1, t0:t0+NT], in_=srow)
        peb_cm.__exit__(None, None, None)
        psd_cm.__exit__(None, None, None)
        psc2_cm.__exit__(None, None, None)
        psc_cm.__exit__(None, None, None)
        pcb_cm.__exit__(None, None, None)
        pc_cm.__exit__(None, None, None)
        rqk_cm.__exit__(None, None, None)
        if DBG:
            for k in range(KD):
                nc.sync.dma_start(out=dbg_od[k*128:(k+1)*128, :], in_=odT[k])
            nc.sync.dma_start(out=dbg_sums[:, :], in_=sums_t[:, :])

        # ====== D: softmax-normalize + inner LN stats + proj + resid ======
        rowsB_cm = tc.tile_pool(name="rowsB", bufs=1, side="right")
        rowsB = rowsB_cm.__enter__()
        muo_row = rowsB.tile([1, TP], F32R, name="muo_row")
        muo_b = rowsB.tile([1, TP], BF16, name="muo_b")
        rso_row = rowsB.tile([1, TP], F32R, name="rso_row")
        mu2_row = rowsB.tile([1, TP], F32R, name="mu2_row")
        rs2_row = rowsB.tile([1, TP], F32R, name="rs2_row")
        roh_cm = tc.tile_pool(name="roh", bufs=1, side="right")
        roh = roh_cm.__enter__()
        ohT = [roh.tile([128, TP], BF16, name=f"ohT{k}", tag=f"ohT{k}")
               for k in range(KD)]
        with tc.tile_pool(name="pdo", bufs=3) as pdo, \
             tc.tile_pool(name="prd", bufs=1) as prd, \
             tc.tile_pool(name="pds", bufs=1) as pds, \
             tc.tile_pool(name="psf", bufs=2, space="PSUM") as psf, \
             tc.tile_pool(name="pso", bufs=2, space="PSUM") as pso:
            recip = pds.tile([16, TP], F32R)
            for (toff, tlen) in TOK_SLICES:
                nc.vector.reciprocal(recip[:, toff:toff+tlen],
                                     sums_t[:, toff:toff+tlen])
            sel_t = [pds.tile([16, 128], F32R, name=f"sel{k}", tag=f"sel{k}")
                     for k in range(KD)]
            for k in range(KD):
                nc.sync.dma_start(out=sel_t[k],
                                  in_=sel_d[k, :, :].bitcast(F32R))
            for (toff, tlen) in TOK_SLICES:
                o_sx = pso.tile([1, 512], F32, tag="o_sx")
                o_sxx = pso.tile([1, 512], F32, tag="o_sxx")
                for k in range(KD):
                    fld = psf.tile([128, 512], F32, tag="d_fld")
                    nc.tensor.matmul(fld[:, :tlen], sel_t[k],
                                     recip[:, toff:toff+tlen],
                                     start=True, stop=True)
                    nc.vector.tensor_mul(ohT[k][:, toff:toff+tlen],
                                         odT[k][:, toff:toff+tlen],
                                         fld[:, :tlen])
                    nc.tensor.matmul(o_sx[:, :tlen], ones_cb,
                                     ohT[k][:, toff:toff+tlen],
                                     start=(k == 0), stop=(k == KD-1))
                    sq = pdo.tile([128, 512], F32R, tag="d_sq")
                    nc.scalar.activation(out=sq[:, :tlen],
                                         in_=ohT[k][:, toff:toff+tlen],
                                         func=AF.Square)
                    nc.tensor.matmul(o_sxx[:, :tlen], ones_c, sq[:, :tlen],
                                     start=(k == 0), stop=(k == KD-1))
                ln_rows(prd, o_sx, o_sxx, muo_row, rso_row, toff, tlen, D,
                        tagp="d_")
                nc.vector.tensor_copy(muo_b[:, toff:toff+tlen],
                                      muo_row[:, toff:toff+tlen])
        rod_cm.__exit__(None, None, None)
        rowsA_cm.__exit__(None, None, None)

        # proj + residual + norm2 stats
        rx2_cm = tc.tile_pool(name="rx2", bufs=1)
        rx2 = rx2_cm.__enter__()
        x2all = rx2.tile([128, KD, TP], F32R, name="x2all")
        x2 = [x2all[:, m, :] for m in range(KD)]
        rof = rx2.tile([128, TP], BF16, name="rof")
        with tc.tile_pool(name="pdp", bufs=3) as pdp, \
             tc.tile_pool(name="prp", bufs=1) as prp, \
             tc.tile_pool(name="pdw", bufs=1) as pdw, \
             tc.tile_pool(name="psp", bufs=2, space="PSUM") as psp, \
             tc.tile_pool(name="pss", bufs=2, space="PSUM") as pss:
            wp = [pdw.tile([128, D], BF16, name=f"wp{k}", tag=f"wp{k}")
                  for k in range(KD)]
            for k in range(KD):
                nc.sync.dma_start(out=wp[k], in_=wpT_d[k*128:(k+1)*128, :])
            cp_t = pdw.tile([1, D], BF16)
            nc.sync.dma_start(out=cp_t, in_=cpn[:, :])
            pb_t = pdw.tile([128, KD], F32)
            nc.sync.dma_start(out=pb_t, in_=pb_c[:, :])
            for (toff, tlen) in TOK_SLICES:
                ps_b = psp.tile([128, 512], F32, tag="rof_bc")
                nc.tensor.matmul(ps_b[:, :tlen], ones_r,
                                 rso_row[:, toff:toff+tlen],
                                 start=True, stop=True)
                nc.scalar.activation(out=rof[:, toff:toff+tlen],
                                     in_=ps_b[:, :tlen], func=AF.Copy)
            for (toff, tlen) in TOK_SLICES:
                x_sx = pss.tile([1, 512], F32, tag="x_sx")
                x_sxx = pss.tile([1, 512], F32, tag="x_sxx")
                for mq in range(KD // 4):
                    xs4 = pdp.tile([128, 4, 512], F32, tag="pj_xs")
                    nc.sync.dma_start(
                        out=xs4[:, :, :tlen],
                        in_=xT_d[mq*512:(mq+1)*512,
                                 toff:toff+tlen].rearrange(
                                     "(g p) c -> p g c", p=128))
                    for mj in range(4):
                        m = mq * 4 + mj
                        ps = psp.tile([128, 512], F32, tag="pj_ps")
                        for k in range(KD):
                            nc.tensor.matmul(ps[:, :tlen],
                                             wp[k][:, m*128:(m+1)*128],
                                             ohT[k][:, toff:toff+tlen],
                                             start=(k == 0), stop=False)
                        nc.tensor.matmul(ps[:, :tlen],
                                         cp_t[:, m*128:(m+1)*128],
                                         muo_b[:, toff:toff+tlen],
                                         start=False, stop=True)
                        tt = pdp.tile([128, 512], F32R, tag="pj_t")
                        nc.vector.tensor_mul(tt[:, :tlen], ps[:, :tlen],
                                             rof[:, toff:toff+tlen])
                        nc.vector.scalar_tensor_tensor(
                            out=x2[m][:, toff:toff+tlen], in0=tt[:, :tlen],
                            scalar=pb_t[:, m:m+1], in1=xs4[:, mj, :tlen],
                            op0=ALU.add, op1=ALU.add)
                        nc.tensor.matmul(x_sx[:, :tlen], ones_c,
                                         x2[m][:, toff:toff+tlen],
                                         start=(m == 0), stop=(m == KD-1))
                        sq = pdp.tile([128, 512], F32R, tag="pj_sq")
                        nc.scalar.activation(out=sq[:, :tlen],
                                             in_=x2[m][:, toff:toff+tlen],
                                             func=AF.Square)
                        nc.tensor.matmul(x_sxx[:, :tlen], ones_c,
                                         sq[:, :tlen],
                                         start=(m == 0), stop=(m == KD-1))
                    nc.scalar.dma_start(
                        out=x2d[mq*512:(mq+1)*512,
                                toff:toff+tlen].bitcast(F32R).rearrange(
                                    "(g p) c -> p g c", p=128),
                        in_=x2all[:, mq*4:(mq+1)*4, toff:toff+tlen])
                ln_rows(prp, x_sx, x_sxx, mu2_row, rs2_row, toff, tlen, D,
                        tagp="x2_")
            if DBG:
                for m in range(KD):
                    nc.sync.dma_start(
                        out=dbg_x2[m*128:(m+1)*128, :].bitcast(F32R),
                        in_=x2[m])
        roh_cm.__exit__(None, None, None)

        # ================= E0: xhat2 =================
        rx3_cm = tc.tile_pool(name="rx3", bufs=1, side="right")
        rx3 = rx3_cm.__enter__()
        xh2 = [rx3.tile([128, TP], BF16, name=f"xh2_{k}", tag=f"xh2_{k}")
               for k in range(KD)]
        with tc.tile_pool(name="pe0", bufs=3) as pe0, \
             tc.tile_pool(name="pse0", bufs=2, space="PSUM") as pse0:
            for (toff, tlen) in TOK_SLICES:
                ps_mu = pse0.tile([128, 512], F32, tag="mu2_bc")
                nc.tensor.matmul(ps_mu[:, :tlen], ones_r,
                                 mu2_row[:, toff:toff+tlen],
                                 start=True, stop=True)
                mu2f = pe0.tile([128, 512], F32, tag="mu2f")
                nc.scalar.activation(out=mu2f[:, :tlen], in_=ps_mu[:, :tlen],
                                     func=AF.Copy)
                ps_rs = pse0.tile([128, 512], F32, tag="rs2_bc")
                nc.tensor.matmul(ps_rs[:, :tlen], ones_r,
                                 rs2_row[:, toff:toff+tlen],
                                 start=True, stop=True)
                rs2f = pe0.tile([128, 512], BF16, tag="rs2f")
                nc.scalar.activation(out=rs2f[:, :tlen], in_=ps_rs[:, :tlen],
                                     func=AF.Copy)
                for k in range(KD):
                    df = pe0.tile([128, 512], BF16, tag="e0_d")
                    nc.vector.tensor_sub(df[:, :tlen],
                                         x2[k][:, toff:toff+tlen],
                                         mu2f[:, :tlen])
                    nc.vector.tensor_mul(xh2[k][:, toff:toff+tlen],
                                         df[:, :tlen], rs2f[:, :tlen])
            if DBG:
                for k in range(KD):
                    nc.sync.dma_start(out=dbg_xh2[k*128:(k+1)*128, :],
                                      in_=xh2[k])
        rx2_cm.__exit__(None, None, None)

        # w3 prefetch (overlaps E): loads don't depend on E outputs
        pgw_cm = tc.tile_pool(name="pgw", bufs=1)
        pgw = pgw_cm.__enter__()
        w3 = [pgw.tile([128, D], BF16, name=f"w3_{mh}", tag=f"w3_{mh}")
              for mh in range(MH)]
        for mh in range(MH):
            (nc.sync, nc.scalar)[mh % 2].dma_start(
                out=w3[mh], in_=w3T_d[mh*128:(mh+1)*128, :])
        c3_t = pgw.tile([1, D], BF16)
        nc.sync.dma_start(out=c3_t, in_=c3n[:, :])
        w3b_t = pgw.tile([128, KD], F32)
        nc.sync.dma_start(out=w3b_t, in_=w3b_c[:, :])

        # ================= E: w1/w2 + silu -> hd =================
        with tc.tile_pool(name="pe", bufs=3, side="right") as pe, \
             tc.tile_pool(name="pew", bufs=2, side="right") as pew, \
             tc.tile_pool(name="pec", bufs=1, side="right") as pec, \
             tc.tile_pool(name="pse1", bufs=3, space="PSUM", side="right") as pse1, \
             tc.tile_pool(name="pse2", bufs=3, space="PSUM", side="right") as pse2:
            w1b_t = pec.tile([128, MH], F32)
            w2b_t = pec.tile([128, MH], F32)
            nc.sync.dma_start(out=w1b_t, in_=w1b_c[:, :])
            nc.sync.dma_start(out=w2b_t, in_=w2b_c[:, :])
            for m in range(MH):
                w1a = pew.tile([128, D], BF16, tag="w1a")
                w2a = pew.tile([128, D], BF16, tag="w2a")
                _we1 = (nc.sync, nc.gpsimd)[m % 2]
                _we2 = (nc.gpsimd, nc.sync)[m % 2]
                _we1.dma_start(
                    out=w1a.rearrange("p (g c) -> p g c", g=KD),
                    in_=w1T_d[:, m*128:(m+1)*128].rearrange(
                        "(g p) c -> p g c", p=128))
                _we2.dma_start(
                    out=w2a.rearrange("p (g c) -> p g c", g=KD),
                    in_=w2T_d[:, m*128:(m+1)*128].rearrange(
                        "(g p) c -> p g c", p=128))
                w1m = [w1a[:, k*128:(k+1)*128] for k in range(KD)]
                w2m = [w2a[:, k*128:(k+1)*128] for k in range(KD)]
                for (toff, tlen) in TOK_SLICES:
                    ps1 = pse1.tile([128, 512], F32, tag="e_ps1")
                    for k in range(KD):
                        nc.tensor.matmul(ps1[:, :tlen], w1m[k],
                                         xh2[k][:, toff:toff+tlen],
                                         start=(k == 0), stop=(k == KD-1))
                    h1 = pe.tile([128, 512], BF16, tag="e_h1")
                    nc.scalar.activation(out=h1[:, :tlen], in_=ps1[:, :tlen],
                                         func=AF.Silu, bias=w1b_t[:, m:m+1],
                                         scale=1.0)
                    ps2 = pse2.tile([128, 512], F32, tag="e_ps2")
                    for k in range(KD):
                        nc.tensor.matmul(ps2[:, :tlen], w2m[k],
                                         xh2[k][:, toff:toff+tlen],
                                         start=(k == 0), stop=(k == KD-1))
                    ht = pe.tile([128, 512], BF16, tag="e_ht")
                    nc.vector.scalar_tensor_tensor(
                        out=ht[:, :tlen], in0=ps2[:, :tlen],
                        scalar=w2b_t[:, m:m+1], in1=h1[:, :tlen],
                        op0=ALU.add, op1=ALU.mult)
                    nc.gpsimd.dma_start(out=hd[m // 4, :, m % 4,
                                               toff:toff+tlen],
                                        in_=ht[:, :tlen])
        rx3_cm.__exit__(None, None, None)
        rowsB_cm.__exit__(None, None, None)

        # ============ G: ffn-LN stats (JIT) + w3 + resid -> outT ============
        with tc.tile_pool(name="pg", bufs=3) as pg, \
             tc.tile_pool(name="prg", bufs=1) as prg, \
             tc.tile_pool(name="pgh", bufs=2) as pgh, \
             tc.tile_pool(name="psg", bufs=1, space="PSUM") as psg, \
             tc.tile_pool(name="psh", bufs=1, space="PSUM") as psh:
            _gq = [nc.sync, nc.scalar, nc.gpsimd, nc.sync]
            for (toff, tlen) in TOK_SLICES:
                hts = []
                for g4 in range(MH // 4):
                    htl = pgh.tile([128, 4, 512], BF16, tag=f"g_h{g4}")
                    _gq[g4 % 4].dma_start(out=htl[:, :, :tlen],
                                          in_=hd[g4, :, :, toff:toff+tlen])
                    for j in range(4):
                        hts.append(htl[:, j, :])
                h_sx = psh.tile([1, 512], F32, tag="h_sx")
                h_sxx = psh.tile([1, 512], F32, tag="h_sxx")
                psA = [psg.tile([128, 512], F32, name=f"g_psA{m}", tag=f"g_ps{m}")
                       for m in range(4)]
                for mh in range(MH):
                    for m in range(4):
                        nc.tensor.matmul(psA[m][:, :tlen], w3[mh][:, m*128:(m+1)*128],
                                         hts[mh][:, :tlen],
                                         start=(mh == 0), stop=False)
                    nc.tensor.matmul(h_sx[:, :tlen], ones_cb, hts[mh][:, :tlen],
                                     start=(mh == 0), stop=(mh == MH-1))
                    sqh = pg.tile([128, 512], F32R, tag="g_sq")
                    nc.scalar.activation(out=sqh[:, :tlen],
                                         in_=hts[mh][:, :tlen], func=AF.Square)
                    nc.tensor.matmul(h_sxx[:, :tlen], ones_c, sqh[:, :tlen],
                                     start=(mh == 0), stop=(mh == MH-1))
                muh_sl = prg.tile([1, 512], F32R, tag="g_muh")
                rsh_sl = prg.tile([1, 512], F32R, tag="g_rsh")
                ln_rows(prg, h_sx, h_sxx, muh_sl, rsh_sl, 0, tlen, HID,
                        tagp="g_")
                muh_b = prg.tile([1, 512], BF16, tag="g_muhb")
                nc.vector.tensor_copy(muh_b[:, :tlen], muh_sl[:, :tlen])
                ps_rh = psh.tile([128, 512], F32, tag="rh_bc")
                nc.tensor.matmul(ps_rh[:, :tlen], ones_r, rsh_sl[:, :tlen],
                                 start=True, stop=True)
                rhf = pg.tile([128, 512], BF16, tag="g_rhf")
                nc.scalar.activation(out=rhf[:, :tlen], in_=ps_rh[:, :tlen],
                                     func=AF.Copy)

                def drain4(m0, psms):
                    x2s4 = pg.tile([128, 4, 512], F32, tag="g_x2")
                    nc.sync.dma_start(
                        out=x2s4[:, :, :tlen],
                        in_=x2d[m0*128:(m0+4)*128, toff:toff+tlen].rearrange(
                            "(g p) c -> p g c", p=128))
                    ot4 = pg.tile([128, 4, 512], F32R, tag="g_ot")
                    for mj in range(4):
                        m = m0 + mj
                        psm = psms[mj]
                        nc.tensor.matmul(psm[:, :tlen],
                                         c3_t[:, m*128:(m+1)*128],
                                         muh_b[:, :tlen],
                                         start=False, stop=True)
                        tt = pg.tile([128, 512], F32R, tag="g_t")
                        nc.vector.tensor_mul(tt[:, :tlen], psm[:, :tlen],
                                             rhf[:, :tlen])
                        nc.vector.scalar_tensor_tensor(
                            out=ot4[:, mj, :tlen], in0=tt[:, :tlen],
                            scalar=w3b_t[:, m:m+1], in1=x2s4[:, mj, :tlen],
                            op0=ALU.add, op1=ALU.add)
                    nc.scalar.dma_start(
                        out=out[m0*128:(m0+4)*128,
                                toff:toff+tlen].bitcast(F32R).rearrange(
                                    "(g p) c -> p g c", p=128),
                        in_=ot4[:, :, :tlen])

                drain4(0, psA)
                psB = [psg.tile([128, 512], F32, name=f"g_psB{m}", tag=f"g_ps{m}")
                       for m in range(4)]
                for mh in range(MH):
                    for m in range(4):
                        nc.tensor.matmul(psB[m][:, :tlen],
                                         w3[mh][:, (m+4)*128:(m+5)*128],
                                         hts[mh][:, :tlen],
                                         start=(mh == 0), stop=False)
                drain4(4, psB)

        pgw_cm.__exit__(None, None, None)
        consts_cm.__exit__(None, None, None)
    _fix_wait_limits(nc, mybir)
    _CACHE[key] = nc
    return nc


def prep_inputs(inputs):
    f32 = np.float32
    import ml_dtypes
    bf16 = ml_dtypes.bfloat16
    x = np.asarray(inputs['x'], f32)
    g1 = np.asarray(inputs['norm1_g'], f32); b1 = np.asarray(inputs['norm1_b'], f32)
    gi = np.asarray(inputs['inner_ln_g'], f32); bi = np.asarray(inputs['inner_ln_b'], f32)
    g2 = np.asarray(inputs['norm2_g'], f32); b2 = np.asarray(inputs['norm2_b'], f32)
    gf = np.asarray(inputs['ffn_ln_g'], f32); bf = np.asarray(inputs['ffn_ln_b'], f32)
    q_w = np.asarray(inputs['q_w'], f32); q_b = np.asarray(inputs['q_b'], f32)
    k_w = np.asarray(inputs['k_w'], f32)
    v_w = np.asarray(inputs['v_w'], f32); v_b = np.asarray(inputs['v_b'], f32)
    p_w = np.asarray(inputs['proj_w'], f32); p_b = np.asarray(inputs['proj_b'], f32)
    w1_w = np.asarray(inputs['w1_w'], f32); w1_b = np.asarray(inputs['w1_b'], f32)
    w2_w = np.asarray(inputs['w2_w'], f32); w2_b = np.asarray(inputs['w2_b'], f32)
    w3_w = np.asarray(inputs['w3_w'], f32); w3_b = np.asarray(inputs['w3_b'], f32)
    tab = np.asarray(inputs['rel_bias_table'], f32)
    rc = np.asarray(inputs['rope_cos'], f32)
    rs = np.asarray(inputs['rope_sin'], f32)

    wqTe = (q_w.T * g1[:, None] * SCALE).astype(f32)
    qb_eff = ((q_b + q_w @ b1) * SCALE).astype(f32)
    wkTe = (k_w.T * g1[:, None]).astype(f32)
    kb_eff = (k_w @ b1).astype(f32)
    wvTe = (v_w.T * g1[:, None]).astype(f32)
    vb_eff = (v_b + v_w @ b1).astype(f32)
    wpTe = (p_w.T * gi[:, None]).astype(f32)
    pb_eff = (p_b + p_w @ bi).astype(f32)
    w1Te = (w1_w.T * g2[:, None]).astype(f32)
    w1b_eff = (w1_b + w1_w @ b2).astype(f32)
    w2Te = (w2_w.T * g2[:, None]).astype(f32)
    w2b_eff = (w2_b + w2_w @ b2).astype(f32)
    w3Te = (w3_w.T * gf[:, None]).astype(f32)
    w3b_eff = (w3_b + w3_w @ bf).astype(f32)

    cosr = np.ones((128, TP), f32)
    sinr = np.zeros((128, TP), f32)
    for b in range(BL):
        cosr[0:64, b*NT+1:(b+1)*NT] = rc.T
        cosr[64:128, b*NT+1:(b+1)*NT] = rc.T
        sinr[0:64, b*NT+1:(b+1)*NT] = rs.T
        sinr[64:128, b*NT+1:(b+1)*NT] = rs.T

    r2 = np.zeros((64, 64), f32)
    for i in range(32):
        r2[2*i, 2*i+1] = -1.0
        r2[2*i+1, 2*i] = 1.0
    r2b = np.zeros((128, 128), f32)
    r2b[0:64, 0:64] = r2
    r2b[64:128, 64:128] = r2

    idx = _rel_pos_index()
    bias_full = tab[idx, :]                      # [NT(q), NT(k), H]
    expb2 = np.zeros((H // 2, 2, 128, 2, QP), f32)
    expbc = np.zeros((H // 2, 65, QP), f32)
    for h in range(H):
        bh = np.exp(bias_full[:, :, h].T)        # [key, query]
        p, j = h // 2, h % 2
        for kc in range(2):
            expb2[p, kc, :, j, :NT] = bh[kc*128:(kc+1)*128, :]
        expbc[p, 64*j, :NT] = bh[256, :]
    expb2[:, :, :, :, NT:] = 1.0
    expbc[:, 0, NT:] = 1.0
    expbc[:, 64, NT:] = 1.0

    sel = np.zeros((KD, 16, 128), f32)
    for k in range(KD):
        sel[k, 2*k, 0:64] = 1.0
        sel[k, 2*k+1, 64:128] = 1.0

    maskr = np.zeros((1, TP), f32)
    maskr[0, :T] = 1.0

    common = {
        'wqT': wqTe, 'wkT': wkTe, 'wvT': wvTe,
        'wpT': wpTe.astype(bf16), 'w1T': w1Te.astype(bf16),
        'w2T': w2Te.astype(bf16), 'w3T': w3Te.astype(bf16),
        'cpn': (-wpTe.astype(bf16).astype(f32).sum(0, keepdims=True)).astype(bf16),
        'c3n': (-w3Te.astype(bf16).astype(f32).sum(0, keepdims=True)).astype(bf16),
        'qb_c': qb_eff.reshape(KD, 128).T.copy(),
        'kb_c': kb_eff.reshape(KD, 128).T.copy(),
        'pb_c': pb_eff.reshape(KD, 128).T.copy(),
        'w1b_c': w1b_eff.reshape(MH, 128).T.copy(),
        'w2b_c': w2b_eff.reshape(MH, 128).T.copy(),
        'w3b_c': w3b_eff.reshape(KD, 128).T.copy(),
        'vbr': vb_eff.reshape(1, D).astype(bf16),
        'cosr': cosr.astype(bf16), 'sinr': sinr.astype(bf16),
        'r2T': r2b.T.copy().astype(bf16),
        'expb2': expb2.transpose(2, 0, 1, 3, 4).reshape(
            128, (H // 2) * 2 * 2 * QP).copy().astype(bf16),
        'expbc': expbc.transpose(1, 0, 2).reshape(
            65, (H // 2) * QP).copy().astype(bf16), 'sel': sel,
        'onesc': np.ones((128, 1), f32), 'onesr': np.ones((1, 128), f32),
        'onespat': np.ones((128, 16), f32).astype(bf16),
        'maskr': maskr,
    }
    in_maps = []
    for c in range(NCORES):
        xc = np.zeros((TP, D), f32)
        xc[:T] = x[c*BL:(c+1)*BL].reshape(T, D)
        im = dict(common)
        im['xT'] = np.ascontiguousarray(xc.T)
        in_maps.append(im)
    return in_maps


def postprocess_out(full):
    """full: concatenated per-core 'out' arrays, shape (8*D, TP)."""
    full = np.asarray(full).reshape(NCORES, D, TP)
    outs = [full[c].T[:T].reshape(BL, NT, D) for c in range(NCORES)]
    return np.concatenate(outs, 0).astype(np.float32)


def kernel(**inputs):
    from concourse.bass_utils import run_bass_kernel_spmd
    nc = build_module()
    in_maps = prep_inputs(inputs)
    res = run_bass_kernel_spmd(nc, in_maps, list(range(NCORES)))
    outs = []
    for c in range(NCORES):
        outs.append(res.results[c]['out'].T[:T].reshape(BL, NT, D))
    return np.concatenate(outs, 0).astype(np.float32)

